# revision 10
# baseline (speedup 1.0000x reference)
"""BitFFN (ternary-quantized MLP) Trainium2 kernel, data-parallel over 8 NeuronCores.

Computation (matches the fp32 reference):
    w_q   = sign(w) * (|w| >= 0.7 * mean(|w|))          for w1 and w2
    h     = gelu(x @ w1_q.T + b1)                        [B*S, d_ff]
    out   = h @ w2_q.T + b2                              [B*S, d_model]

Strategy: pure data-parallel over the B*S=16384 rows (2048 rows/core); weights
replicated per core. On device, per core:
  - scale phase: each core abs-sums its 1/8 slice of w1/w2, partial sums are
    AllReduce'd (2 floats), thresholds broadcast to [128,1] via tiny matmuls.
  - fc1: composable tiled matmul hT[f,m] = w1qT.T-contraction with ternarization
    of w1 fused into the kxm producer (fp32 compare -> exact bf16 {-1,0,1});
    PSUM eviction applies gelu(+b1) and stores hT as bf16 to DRAM.
  - w2 quant pass (overlaps fc1 on DVE/DMA): w2 -> ternary bf16 in DRAM.
  - fc2: outT[d,m] = contraction over d_ff of w2qT x hT; eviction adds b2.
Host does layout-only work: transposes/casts for DMA-friendly layouts and the
final gather/transpose back to [4, 4096, 2048].
"""
import os
from contextlib import ExitStack

import ml_dtypes
import numpy as np

import concourse.mybir as mybir
import concourse.tile as tile
from concourse import bacc
from concourse.bass_utils import run_bass_kernel_spmd
from concourse.kernels.tile_matmul import (
    composable_matmul_tile_kernel,
    dma_from_dram_kxm,
    dma_from_dram_kxn,
    dma_to_dram_mxn,
)

F32 = mybir.dt.float32
BF16 = mybir.dt.bfloat16
P = 128
D_MODEL = 2048
D_FF = 8192
N_CORES = 8
M_TOTAL = 4 * 4096
M_CORE = M_TOTAL // N_CORES  # 2048 rows per core
N_W = D_FF * D_MODEL  # elements per weight matrix
SLICE_F = N_W // N_CORES // P  # 16384: per-core scale slice is [128, SLICE_F]

_BUILD_CACHE = {}


def _build_nc():
    nc = bacc.Bacc("TRN2", target_bir_lowering=False, debug=False, num_devices=N_CORES)

    xT = nc.declare_dram_parameter("xT", [D_MODEL, M_CORE], BF16, isOutput=False)
    w1T = nc.declare_dram_parameter("w1T", [D_MODEL, D_FF], F32, isOutput=False)
    w2T = nc.declare_dram_parameter("w2T", [D_FF, D_MODEL], F32, isOutput=False)
    w1s = nc.declare_dram_parameter("w1s", [P, SLICE_F], F32, isOutput=False)
    w2s = nc.declare_dram_parameter("w2s", [P, SLICE_F], F32, isOutput=False)
    b1 = nc.declare_dram_parameter("b1", [D_FF], F32, isOutput=False)
    b2 = nc.declare_dram_parameter("b2", [D_MODEL], F32, isOutput=False)
    outT = nc.declare_dram_parameter("outT", [D_MODEL, M_CORE], F32, isOutput=True)

    GELU = mybir.ActivationFunctionType.Gelu
    COPY = mybir.ActivationFunctionType.Copy
    IS_GE = mybir.AluOpType.is_ge
    IS_LE = mybir.AluOpType.is_le
    ADD = mybir.AluOpType.add
    AX = mybir.AxisListType.X

    with tile.TileContext(nc) as tc, ExitStack() as top:
        const = top.enter_context(tc.tile_pool(name="const", bufs=1))
        dram = top.enter_context(tc.tile_pool(name="dram", bufs=1, space="DRAM"))

        b1_sb = const.tile([P, D_FF // P], F32)
        nc.sync.dma_start(out=b1_sb[:], in_=b1.ap().rearrange("(a p) -> p a", p=P))
        b2_sb = const.tile([P, D_MODEL // P], F32)
        nc.sync.dma_start(out=b2_sb[:], in_=b2.ap().rearrange("(a p) -> p a", p=P))

        thr_pos = const.tile([P, 2], F32)
        thr_neg = const.tile([P, 2], F32)

        # ---------------- scale phase ----------------
        # Each core reduces its own 1/8 slice of |w1| and |w2|; AllReduce the
        # two partial sums; thresholds = 0.7 * sum / N_W broadcast per-partition.
        with (
            tc.tile_pool(name="sstage", bufs=3) as sstage,
            tc.tile_pool(name="sps", bufs=1, space="PSUM") as sps,
        ):
            CH = 4096
            NCH = SLICE_F // CH  # 4 chunks per weight matrix
            acc = const.tile([P, 2 * NCH], F32)
            for wi, wap in enumerate((w1s, w2s)):
                for i in range(NCH):
                    t = sstage.tile([P, CH], F32)
                    nc.sync.dma_start(out=t[:], in_=wap.ap()[:, i * CH : (i + 1) * CH])
                    j = wi * NCH + i
                    nc.vector.tensor_reduce(
                        acc[:, j : j + 1],
                        t[:],
                        axis=AX,
                        op=ADD,
                        apply_absolute_value=True,
                    )
            red = const.tile([P, 2], F32)
            nc.vector.tensor_reduce(red[:, 0:1], acc[:, 0:NCH], axis=AX, op=ADD)
            nc.vector.tensor_reduce(red[:, 1:2], acc[:, NCH : 2 * NCH], axis=AX, op=ADD)

            ones = const.tile([P, 1], F32)
            nc.any.memset(ones[:], 1.0)
            part_ps = sps.tile([2, 1], F32)
            nc.tensor.matmul(part_ps[:], red[:], ones[:], start=True, stop=True)
            part_sb = const.tile([2, 1], F32)
            nc.any.tensor_copy(out=part_sb[:], in_=part_ps[:])

            cc_in = dram.tile([1, 2], F32)
            cc_out = dram.tile([1, 2], F32)
            nc.sync.dma_start(out=cc_in[:], in_=part_sb[:])
            if os.environ.get("BITFFN_NO_CC"):
                # single-core timeline-sim mode: stub the collective
                nc.sync.dma_start(out=cc_out[:], in_=cc_in[:])
            else:
                nc.gpsimd.collective_compute(
                    "AllReduce",
                    ADD,
                    replica_groups=[list(range(N_CORES))],
                    ins=[cc_in[:]],
                    outs=[cc_out[:]],
                )
            tot_sb = const.tile([1, 2], F32)
            nc.sync.dma_start(out=tot_sb[:], in_=cc_out[:])

            coef = const.tile([1, P], F32)
            nc.any.memset(coef[:], 0.7 / N_W)
            thr_ps = sps.tile([P, 2], F32)
            nc.tensor.matmul(thr_ps[:], coef[:], tot_sb[:], start=True, stop=True)
            nc.any.tensor_copy(out=thr_pos[:], in_=thr_ps[:])
            nc.vector.tensor_scalar_mul(thr_neg[:], thr_pos[:], -1.0)

        t1p, t1n = thr_pos[:, 0:1], thr_neg[:, 0:1]
        t2p, t2n = thr_pos[:, 1:2], thr_neg[:, 1:2]

        hT = dram.tile([D_FF, M_CORE], BF16)
        w2qT = dram.tile([D_FF, D_MODEL], BF16)

        # ---------------- fc1 (+ interleaved w2 quant) ----------------
        with ExitStack() as fc1_scope:
            stage = fc1_scope.enter_context(tc.tile_pool(name="kxm_stage", bufs=3))
            kxmq = fc1_scope.enter_context(tc.tile_pool(name="kxmq", bufs=6))
            qtmp = fc1_scope.enter_context(tc.tile_pool(name="qtmp", bufs=3))
            kxn1 = fc1_scope.enter_context(tc.tile_pool(name="kxn1", bufs=6))
            w2stage = fc1_scope.enter_context(tc.tile_pool(name="w2stage", bufs=3))
            w2tmp = fc1_scope.enter_context(tc.tile_pool(name="w2tmp", bufs=3))
            w2qsb = fc1_scope.enter_context(tc.tile_pool(name="w2qsb", bufs=3))

            # w2 ternarization -> DRAM. Emitted one block per fc1 kxm-producer
            # call so its DVE work interleaves 1:1 with fc1's w1 quant instead
            # of hogging the engine up front or trailing behind fc1.
            w2_state = {"blk": 0}

            def emit_w2_block():
                blk = w2_state["blk"]
                if blk >= D_FF // P:
                    return
                w2_state["blk"] = blk + 1
                s = w2stage.tile([P, D_MODEL], F32)
                nc.sync.dma_start(out=s[:], in_=w2T.ap()[blk * P : (blk + 1) * P, :])
                q = w2qsb.tile([P, D_MODEL], BF16)
                a = w2tmp.tile([P, D_MODEL], BF16)
                nc.vector.tensor_scalar(q[:], s[:], t2p, None, IS_GE)
                nc.vector.tensor_scalar(a[:], s[:], t2n, None, IS_LE)
                nc.vector.tensor_sub(q[:], q[:], a[:])
                nc.sync.dma_start(out=w2qT[blk * P : (blk + 1) * P, :], in_=q[:])

            base_producer, kxm_shape = dma_from_dram_kxm(stage, w1T.ap())

            def kxm_q_producer(nc_, md):
                t32 = base_producer(nc_, md)
                q = kxmq.tile([P, md.k_subtiles, md.m_tile], BF16, tag="kxmq")
                a = qtmp.tile([P, md.k_subtiles, md.m_tile], BF16, tag="qtmp")
                nc_.vector.tensor_scalar(q[:], t32[:], t1p, None, IS_GE)
                nc_.vector.tensor_scalar(a[:], t32[:], t1n, None, IS_LE)
                nc_.vector.tensor_sub(q[:], q[:], a[:])
                emit_w2_block()
                return q

            kxn_producer, kxn_shape = dma_from_dram_kxn(kxn1, xT.ap())

            def fc1_reducer(nc_, psum, sbuf, md):
                j = md.m_tile_idx * md.m_subtiles + md.m_subtile_idx
                nc_.scalar.activation(sbuf, psum, GELU, bias=b1_sb[:, j : j + 1])

            composable_matmul_tile_kernel(
                tc=tc,
                kxm_shape=kxm_shape,
                kxn_shape=kxn_shape,
                output_type=BF16,
                kxm_producer=kxm_q_producer,
                kxn_producer=kxn_producer,
                mxn_consumer=dma_to_dram_mxn(hT[:]),
                mxn_subtile_reducer=fc1_reducer,
                psum_n_bufs=2,
            )

            # drain any w2 blocks not covered by producer calls
            while w2_state["blk"] < D_FF // P:
                emit_w2_block()

        # ---------------- fc2 ----------------
        with ExitStack() as fc2_scope:
            kxm2 = fc2_scope.enter_context(tc.tile_pool(name="kxm2", bufs=18))
            kxn2 = fc2_scope.enter_context(tc.tile_pool(name="kxn2", bufs=18))

            kxm2_producer, kxm2_shape = dma_from_dram_kxm(kxm2, w2qT[:])
            kxn2_producer, kxn2_shape = dma_from_dram_kxn(kxn2, hT[:])

            def fc2_reducer(nc_, psum, sbuf, md):
                j = md.m_tile_idx * md.m_subtiles + md.m_subtile_idx
                nc_.any.tensor_scalar_add(sbuf, psum, b2_sb[:, j : j + 1])

            composable_matmul_tile_kernel(
                tc=tc,
                kxm_shape=kxm2_shape,
                kxn_shape=kxn2_shape,
                output_type=F32,
                kxm_producer=kxm2_producer,
                kxn_producer=kxn2_producer,
                mxn_consumer=dma_to_dram_mxn(outT.ap()),
                mxn_subtile_reducer=fc2_reducer,
                psum_n_bufs=2,
            )

    nc.compile()
    return nc


def _get_nc():
    if "nc" not in _BUILD_CACHE:
        _BUILD_CACHE["nc"] = _build_nc()
    return _BUILD_CACHE["nc"]


def _prepare_in_maps(x, w1, b1, w2, b2):
    x = np.asarray(x, dtype=np.float32)
    w1 = np.asarray(w1, dtype=np.float32)
    w2 = np.asarray(w2, dtype=np.float32)
    b1 = np.asarray(b1, dtype=np.float32)
    b2 = np.asarray(b2, dtype=np.float32)

    x2 = x.reshape(M_TOTAL, D_MODEL)
    w1T = np.ascontiguousarray(w1.T)  # [D_MODEL, D_FF] f32
    w2T = np.ascontiguousarray(w2.T)  # [D_FF, D_MODEL] f32
    w1sl = w1.reshape(N_CORES, P, SLICE_F)
    w2sl = w2.reshape(N_CORES, P, SLICE_F)

    in_maps = []
    for c in range(N_CORES):
        shard = x2[c * M_CORE : (c + 1) * M_CORE]
        xT_c = np.ascontiguousarray(shard.T).astype(ml_dtypes.bfloat16)
        in_maps.append(
            {
                "xT": xT_c,
                "w1T": w1T,
                "w2T": w2T,
                "w1s": w1sl[c],
                "w2s": w2sl[c],
                "b1": b1,
                "b2": b2,
            }
        )
    return in_maps


def _assemble(res):
    outT_full = np.concatenate(
        [res.results[c]["outT"] for c in range(N_CORES)], axis=1
    )  # [D_MODEL, M_TOTAL]
    out = np.ascontiguousarray(outT_full.T).reshape(4, 4096, D_MODEL)
    return out.astype(np.float32, copy=False)


def kernel(x, w1, b1, w2, b2):
    nc = _get_nc()
    in_maps = _prepare_in_maps(x, w1, b1, w2, b2)
    res = run_bass_kernel_spmd(nc, in_maps, list(range(N_CORES)))
    return _assemble(res)


if __name__ == "__main__":
    # quick self-exercise with random data
    rng = np.random.default_rng(0)
    x = rng.standard_normal((4, 4096, D_MODEL), dtype=np.float32)
    w1 = rng.standard_normal((D_FF, D_MODEL), dtype=np.float32)
    w2 = rng.standard_normal((D_MODEL, D_FF), dtype=np.float32)
    out = kernel(x=x, w1=w1, b1=np.zeros(D_FF, np.float32), w2=w2, b2=np.zeros(D_MODEL, np.float32))
    print(out.shape, out.dtype)


# revision 13
# speedup vs baseline: 39.5171x; 39.5171x over previous
"""BitFFN (ternary-quantized MLP) Trainium2 kernel, data-parallel over 8 NeuronCores.

Computation (matches the fp32 reference):
    w_q   = sign(w) * (|w| >= 0.7 * mean(|w|))          for w1 and w2
    h     = gelu(x @ w1_q.T + b1)                        [B*S, d_ff]
    out   = h @ w2_q.T + b2                              [B*S, d_model]

Strategy: pure data-parallel over the B*S=16384 rows (2048 rows/core); weights
replicated per core. On device, per core:
  - scale phase: each core abs-sums its 1/8 slice of w1/w2, partial sums are
    AllReduce'd (2 floats), thresholds broadcast to [128,1] via tiny matmuls.
  - fc1: composable tiled matmul hT[f,m] with ternarization of w1 fused into
    the kxm producer (fp32 compare -> exact fp16 {-1,0,1}); PSUM eviction
    applies gelu(+b1) (exact-erf ACT table) and stores hT as fp16 to DRAM.
  - w2 quant blocks are emitted interleaved with fc1's kxm producer calls so
    their DVE work fills gaps under fc1's matmuls; result goes to DRAM.
  - fc2: outT[d,m] = contraction over d_ff of w2qT x hT; eviction adds b2.
Matmuls run in fp16 (same PE rate as bf16, 8x finer rounding; ternary weights
are exact). Host does layout-only work: transposes/casts for DMA-friendly
layouts and the final gather/transpose back to [4, 4096, 2048].

`repeats` unrolls the whole pipeline N times in one NEFF — used by test.py to
measure marginal device time free of dispatch overhead; the graded path uses
repeats=1.
"""
import os
from contextlib import ExitStack

import numpy as np

import concourse.mybir as mybir
import concourse.tile as tile
from concourse import bacc
from concourse.bass_utils import run_bass_kernel_spmd
from concourse.kernels.tile_matmul import (
    composable_matmul_tile_kernel,
    dma_from_dram_kxm,
    dma_from_dram_kxn,
    dma_to_dram_mxn,
)

F32 = mybir.dt.float32
HALF = mybir.dt.float16  # same PE rate as bf16, 10 mantissa bits
P = 128
D_MODEL = 2048
D_FF = 8192
N_CORES = 8
M_TOTAL = 4 * 4096
M_CORE = M_TOTAL // N_CORES  # 2048 rows per core
N_W = D_FF * D_MODEL  # elements per weight matrix
SLICE_F = N_W // N_CORES // P  # 16384: per-core scale slice is [128, SLICE_F]

GELU = mybir.ActivationFunctionType.Gelu
IS_GE = mybir.AluOpType.is_ge
IS_LE = mybir.AluOpType.is_le
ADD = mybir.AluOpType.add
AX = mybir.AxisListType.X

_BUILD_CACHE = {}


def _emit_scale_phase(nc, tc, ios, const, dram, thr_pos, thr_neg, rep):
    """Per-core partial |w| sums -> AllReduce -> per-partition thresholds."""
    with (
        tc.tile_pool(name=f"sstage{rep}", bufs=3) as sstage,
        tc.tile_pool(name=f"sps{rep}", bufs=1, space="PSUM") as sps,
    ):
        CH = 4096
        NCH = SLICE_F // CH  # 4 chunks per weight matrix
        acc = const.tile([P, 2 * NCH], F32, tag=f"acc{rep}")
        for wi, wap in enumerate((ios["w1s"], ios["w2s"])):
            for i in range(NCH):
                t = sstage.tile([P, CH], F32)
                nc.sync.dma_start(out=t[:], in_=wap.ap()[:, i * CH : (i + 1) * CH])
                j = wi * NCH + i
                nc.vector.tensor_reduce(
                    acc[:, j : j + 1], t[:], axis=AX, op=ADD,
                    apply_absolute_value=True,
                )
        red = const.tile([P, 2], F32, tag=f"red{rep}")
        nc.vector.tensor_reduce(red[:, 0:1], acc[:, 0:NCH], axis=AX, op=ADD)
        nc.vector.tensor_reduce(red[:, 1:2], acc[:, NCH : 2 * NCH], axis=AX, op=ADD)

        ones = const.tile([P, 1], F32, tag=f"ones{rep}")
        nc.any.memset(ones[:], 1.0)
        part_ps = sps.tile([2, 1], F32)
        nc.tensor.matmul(part_ps[:], red[:], ones[:], start=True, stop=True)
        part_sb = const.tile([2, 1], F32, tag=f"part{rep}")
        nc.any.tensor_copy(out=part_sb[:], in_=part_ps[:])

        cc_in = dram.tile([1, 2], F32, tag=f"ccin{rep}")
        cc_out = dram.tile([1, 2], F32, tag=f"ccout{rep}")
        nc.sync.dma_start(out=cc_in[:], in_=part_sb[:])
        if os.environ.get("BITFFN_NO_CC"):
            # single-core timeline-sim mode: stub the collective
            nc.sync.dma_start(out=cc_out[:], in_=cc_in[:])
        else:
            nc.gpsimd.collective_compute(
                "AllReduce",
                ADD,
                replica_groups=[list(range(N_CORES))],
                ins=[cc_in[:]],
                outs=[cc_out[:]],
            )
        tot_sb = const.tile([1, 2], F32, tag=f"tot{rep}")
        nc.sync.dma_start(out=tot_sb[:], in_=cc_out[:])

        coef = const.tile([1, P], F32, tag=f"coef{rep}")
        nc.any.memset(coef[:], 0.7 / N_W)
        thr_ps = sps.tile([P, 2], F32)
        nc.tensor.matmul(thr_ps[:], coef[:], tot_sb[:], start=True, stop=True)
        nc.any.tensor_copy(out=thr_pos[:], in_=thr_ps[:])
        nc.vector.tensor_scalar_mul(thr_neg[:], thr_pos[:], -1.0)


def _emit_pipeline(nc, tc, ios, const, dram, b1_sb, b2_sb, rep):
    thr_pos = const.tile([P, 2], F32, tag=f"thrp{rep}")
    thr_neg = const.tile([P, 2], F32, tag=f"thrn{rep}")
    _emit_scale_phase(nc, tc, ios, const, dram, thr_pos, thr_neg, rep)
    t1p, t1n = thr_pos[:, 0:1], thr_neg[:, 0:1]
    t2p, t2n = thr_pos[:, 1:2], thr_neg[:, 1:2]

    hT = dram.tile([D_FF, M_CORE], HALF, tag=f"hT{rep}")
    w2qT = dram.tile([D_FF, D_MODEL], HALF, tag=f"w2qT{rep}")

    # ---------------- fc1 (+ interleaved w2 quant) ----------------
    with ExitStack() as fc1_scope:
        stage = fc1_scope.enter_context(tc.tile_pool(name=f"kxm_stage{rep}", bufs=3))
        kxmq = fc1_scope.enter_context(tc.tile_pool(name=f"kxmq{rep}", bufs=6))
        qtmp = fc1_scope.enter_context(tc.tile_pool(name=f"qtmp{rep}", bufs=3))
        kxn1 = fc1_scope.enter_context(tc.tile_pool(name=f"kxn1{rep}", bufs=6))
        w2stage = fc1_scope.enter_context(tc.tile_pool(name=f"w2stage{rep}", bufs=3))
        w2tmp = fc1_scope.enter_context(tc.tile_pool(name=f"w2tmp{rep}", bufs=3))
        w2qsb = fc1_scope.enter_context(tc.tile_pool(name=f"w2qsb{rep}", bufs=3))

        # w2 ternarization -> DRAM. One block per fc1 kxm-producer call so its
        # DVE work interleaves 1:1 with fc1's w1 quant instead of hogging the
        # engine up front or trailing behind fc1.
        w2_state = {"blk": 0}

        def emit_w2_block():
            blk = w2_state["blk"]
            if blk >= D_FF // P:
                return
            w2_state["blk"] = blk + 1
            s = w2stage.tile([P, D_MODEL], F32)
            nc.sync.dma_start(out=s[:], in_=ios["w2T"].ap()[blk * P : (blk + 1) * P, :])
            q = w2qsb.tile([P, D_MODEL], HALF)
            a = w2tmp.tile([P, D_MODEL], HALF)
            nc.vector.tensor_scalar(q[:], s[:], t2p, None, IS_GE)
            nc.vector.tensor_scalar(a[:], s[:], t2n, None, IS_LE)
            nc.vector.tensor_sub(q[:], q[:], a[:])
            nc.sync.dma_start(out=w2qT[blk * P : (blk + 1) * P, :], in_=q[:])

        base_producer, kxm_shape = dma_from_dram_kxm(stage, ios["w1T"].ap())

        def kxm_q_producer(nc_, md):
            t32 = base_producer(nc_, md)
            q = kxmq.tile([P, md.k_subtiles, md.m_tile], HALF, tag="kxmq")
            a = qtmp.tile([P, md.k_subtiles, md.m_tile], HALF, tag="qtmp")
            nc_.vector.tensor_scalar(q[:], t32[:], t1p, None, IS_GE)
            nc_.vector.tensor_scalar(a[:], t32[:], t1n, None, IS_LE)
            nc_.vector.tensor_sub(q[:], q[:], a[:])
            emit_w2_block()
            return q

        kxn_producer, kxn_shape = dma_from_dram_kxn(kxn1, ios["xT"].ap())

        def fc1_reducer(nc_, psum, sbuf, md):
            j = md.m_tile_idx * md.m_subtiles + md.m_subtile_idx
            nc_.scalar.activation(sbuf, psum, GELU, bias=b1_sb[:, j : j + 1])

        composable_matmul_tile_kernel(
            tc=tc,
            kxm_shape=kxm_shape,
            kxn_shape=kxn_shape,
            output_type=HALF,
            kxm_producer=kxm_q_producer,
            kxn_producer=kxn_producer,
            mxn_consumer=dma_to_dram_mxn(hT[:]),
            mxn_subtile_reducer=fc1_reducer,
            psum_n_bufs=2,
        )

        # drain any w2 blocks not covered by producer calls
        while w2_state["blk"] < D_FF // P:
            emit_w2_block()

    # ---------------- fc2 ----------------
    with ExitStack() as fc2_scope:
        kxm2 = fc2_scope.enter_context(tc.tile_pool(name=f"kxm2{rep}", bufs=18))
        kxn2 = fc2_scope.enter_context(tc.tile_pool(name=f"kxn2{rep}", bufs=18))

        kxm2_producer, kxm2_shape = dma_from_dram_kxm(kxm2, w2qT[:])
        kxn2_producer, kxn2_shape = dma_from_dram_kxn(kxn2, hT[:])

        def fc2_reducer(nc_, psum, sbuf, md):
            j = md.m_tile_idx * md.m_subtiles + md.m_subtile_idx
            nc_.any.tensor_scalar_add(sbuf, psum, b2_sb[:, j : j + 1])

        composable_matmul_tile_kernel(
            tc=tc,
            kxm_shape=kxm2_shape,
            kxn_shape=kxn2_shape,
            output_type=F32,
            kxm_producer=kxm2_producer,
            kxn_producer=kxn2_producer,
            mxn_consumer=dma_to_dram_mxn(ios["outT"].ap()),
            mxn_subtile_reducer=fc2_reducer,
            psum_n_bufs=2,
        )


def _build_nc(repeats=1):
    nc = bacc.Bacc("TRN2", target_bir_lowering=False, debug=False, num_devices=N_CORES)

    ios = {
        "xT": nc.declare_dram_parameter("xT", [D_MODEL, M_CORE], HALF, isOutput=False),
        "w1T": nc.declare_dram_parameter("w1T", [D_MODEL, D_FF], F32, isOutput=False),
        "w2T": nc.declare_dram_parameter("w2T", [D_FF, D_MODEL], F32, isOutput=False),
        "w1s": nc.declare_dram_parameter("w1s", [P, SLICE_F], F32, isOutput=False),
        "w2s": nc.declare_dram_parameter("w2s", [P, SLICE_F], F32, isOutput=False),
        "b1": nc.declare_dram_parameter("b1", [D_FF], F32, isOutput=False),
        "b2": nc.declare_dram_parameter("b2", [D_MODEL], F32, isOutput=False),
        "outT": nc.declare_dram_parameter(
            "outT", [D_MODEL, M_CORE], F32, isOutput=True
        ),
    }

    with tile.TileContext(nc) as tc, ExitStack() as top:
        const = top.enter_context(tc.tile_pool(name="const", bufs=1))
        dram = top.enter_context(tc.tile_pool(name="dram", bufs=1, space="DRAM"))

        b1_sb = const.tile([P, D_FF // P], F32)
        nc.sync.dma_start(
            out=b1_sb[:], in_=ios["b1"].ap().rearrange("(a p) -> p a", p=P)
        )
        b2_sb = const.tile([P, D_MODEL // P], F32)
        nc.sync.dma_start(
            out=b2_sb[:], in_=ios["b2"].ap().rearrange("(a p) -> p a", p=P)
        )

        for rep in range(repeats):
            _emit_pipeline(nc, tc, ios, const, dram, b1_sb, b2_sb, rep)

    nc.compile()
    return nc


def _get_nc(repeats=1):
    if repeats not in _BUILD_CACHE:
        _BUILD_CACHE[repeats] = _build_nc(repeats)
    return _BUILD_CACHE[repeats]


def _prepare_in_maps(x, w1, b1, w2, b2):
    x = np.asarray(x, dtype=np.float32)
    w1 = np.asarray(w1, dtype=np.float32)
    w2 = np.asarray(w2, dtype=np.float32)
    b1 = np.asarray(b1, dtype=np.float32)
    b2 = np.asarray(b2, dtype=np.float32)

    x2 = x.reshape(M_TOTAL, D_MODEL)
    w1T = np.ascontiguousarray(w1.T)  # [D_MODEL, D_FF] f32
    w2T = np.ascontiguousarray(w2.T)  # [D_FF, D_MODEL] f32
    w1sl = w1.reshape(N_CORES, P, SLICE_F)
    w2sl = w2.reshape(N_CORES, P, SLICE_F)

    in_maps = []
    for c in range(N_CORES):
        shard = x2[c * M_CORE : (c + 1) * M_CORE]
        xT_c = np.ascontiguousarray(shard.T).astype(np.float16)
        in_maps.append(
            {
                "xT": xT_c,
                "w1T": w1T,
                "w2T": w2T,
                "w1s": w1sl[c],
                "w2s": w2sl[c],
                "b1": b1,
                "b2": b2,
            }
        )
    return in_maps


def _assemble(res):
    outT_full = np.concatenate(
        [res.results[c]["outT"] for c in range(N_CORES)], axis=1
    )  # [D_MODEL, M_TOTAL]
    out = np.ascontiguousarray(outT_full.T).reshape(4, 4096, D_MODEL)
    return out.astype(np.float32, copy=False)


def kernel(x, w1, b1, w2, b2):
    nc = _get_nc()
    in_maps = _prepare_in_maps(x, w1, b1, w2, b2)
    res = run_bass_kernel_spmd(nc, in_maps, list(range(N_CORES)))
    return _assemble(res)


if __name__ == "__main__":
    rng = np.random.default_rng(0)
    x = rng.standard_normal((4, 4096, D_MODEL), dtype=np.float32)
    w1 = rng.standard_normal((D_FF, D_MODEL), dtype=np.float32)
    w2 = rng.standard_normal((D_MODEL, D_FF), dtype=np.float32)
    out = kernel(
        x=x,
        w1=w1,
        b1=np.zeros(D_FF, np.float32),
        w2=w2,
        b2=np.zeros(D_MODEL, np.float32),
    )
    print(out.shape, out.dtype)


# revision 15
# speedup vs baseline: 41.4558x; 1.0491x over previous
"""BitFFN (ternary-quantized MLP) Trainium2 kernel, data-parallel over 8 NeuronCores.

Computation (matches the fp32 reference):
    w_q   = sign(w) * (|w| >= 0.7 * mean(|w|))          for w1 and w2
    h     = gelu(x @ w1_q.T + b1)                        [B*S, d_ff]
    out   = h @ w2_q.T + b2                              [B*S, d_model]

Strategy: pure data-parallel over the B*S=16384 rows (2048 rows/core); weights
replicated per core. On device, per core:
  - scale phase: each core abs-sums its 1/8 slice of w1/w2, partial sums are
    AllReduce'd (2 floats), thresholds broadcast to [128,1] via tiny matmuls.
  - fc1: composable tiled matmul hT[f,m] with ternarization of w1 fused into
    the kxm producer (fp32 compare -> exact fp16 {-1,0,1}); PSUM eviction
    applies gelu(+b1) (exact-erf ACT table) and stores hT as fp16 to DRAM.
  - w2 quant blocks are emitted interleaved with fc1's kxm producer calls so
    their DVE work fills gaps under fc1's matmuls; result goes to DRAM.
  - fc2: outT[d,m] = contraction over d_ff of w2qT x hT; eviction adds b2.
Matmuls run in fp16 (same PE rate as bf16, 8x finer rounding; ternary weights
are exact). Host does layout-only work: transposes/casts for DMA-friendly
layouts and the final gather/transpose back to [4, 4096, 2048].

`repeats` unrolls the whole pipeline N times in one NEFF — used by test.py to
measure marginal device time free of dispatch overhead; the graded path uses
repeats=1.
"""
import os
from contextlib import ExitStack

import numpy as np

import concourse.mybir as mybir
import concourse.tile as tile
from concourse import bacc
from concourse.bass_utils import run_bass_kernel_spmd
from concourse.kernels.tile_matmul import (
    composable_matmul_tile_kernel,
    dma_from_dram_kxm,
    dma_from_dram_kxn,
    dma_to_dram_mxn,
)

F32 = mybir.dt.float32
HALF = mybir.dt.float16  # same PE rate as bf16, 10 mantissa bits
P = 128
D_MODEL = 2048
D_FF = 8192
N_CORES = 8
M_TOTAL = 4 * 4096
M_CORE = M_TOTAL // N_CORES  # 2048 rows per core
N_W = D_FF * D_MODEL  # elements per weight matrix
SLICE_F = N_W // N_CORES // P  # 16384: per-core scale slice is [128, SLICE_F]

GELU = mybir.ActivationFunctionType.Gelu
IS_GE = mybir.AluOpType.is_ge
IS_LE = mybir.AluOpType.is_le
ADD = mybir.AluOpType.add
AX = mybir.AxisListType.X

_BUILD_CACHE = {}


def _emit_scale_phase(nc, tc, ios, const, dram, thr_pos, thr_neg, rep):
    """Per-core partial |w| sums -> AllReduce -> per-partition thresholds."""
    with (
        tc.tile_pool(name=f"sstage{rep}", bufs=3) as sstage,
        tc.tile_pool(name=f"sps{rep}", bufs=1, space="PSUM") as sps,
    ):
        CH = 4096
        NCH = SLICE_F // CH  # 4 chunks per weight matrix
        acc = const.tile([P, 2 * NCH], F32, tag=f"acc{rep}")
        for wi, wap in enumerate((ios["w1s"], ios["w2s"])):
            for i in range(NCH):
                t = sstage.tile([P, CH], F32)
                nc.sync.dma_start(out=t[:], in_=wap.ap()[:, i * CH : (i + 1) * CH])
                j = wi * NCH + i
                nc.vector.tensor_reduce(
                    acc[:, j : j + 1], t[:], axis=AX, op=ADD,
                    apply_absolute_value=True,
                )
        red = const.tile([P, 2], F32, tag=f"red{rep}")
        nc.vector.tensor_reduce(red[:, 0:1], acc[:, 0:NCH], axis=AX, op=ADD)
        nc.vector.tensor_reduce(red[:, 1:2], acc[:, NCH : 2 * NCH], axis=AX, op=ADD)

        ones = const.tile([P, 1], F32, tag=f"ones{rep}")
        nc.any.memset(ones[:], 1.0)
        part_ps = sps.tile([2, 1], F32)
        nc.tensor.matmul(part_ps[:], red[:], ones[:], start=True, stop=True)
        part_sb = const.tile([2, 1], F32, tag=f"part{rep}")
        nc.any.tensor_copy(out=part_sb[:], in_=part_ps[:])

        cc_in = dram.tile([1, 2], F32, tag=f"ccin{rep}")
        cc_out = dram.tile([1, 2], F32, tag=f"ccout{rep}")
        nc.sync.dma_start(out=cc_in[:], in_=part_sb[:])
        if os.environ.get("BITFFN_NO_CC"):
            # single-core timeline-sim mode: stub the collective
            nc.sync.dma_start(out=cc_out[:], in_=cc_in[:])
        else:
            nc.gpsimd.collective_compute(
                "AllReduce",
                ADD,
                replica_groups=[list(range(N_CORES))],
                ins=[cc_in[:]],
                outs=[cc_out[:]],
            )
        tot_sb = const.tile([1, 2], F32, tag=f"tot{rep}")
        nc.sync.dma_start(out=tot_sb[:], in_=cc_out[:])

        coef = const.tile([1, P], F32, tag=f"coef{rep}")
        nc.any.memset(coef[:], 0.7 / N_W)
        thr_ps = sps.tile([P, 2], F32)
        nc.tensor.matmul(thr_ps[:], coef[:], tot_sb[:], start=True, stop=True)
        nc.any.tensor_copy(out=thr_pos[:], in_=thr_ps[:])
        nc.vector.tensor_scalar_mul(thr_neg[:], thr_pos[:], -1.0)


def _emit_pipeline(nc, tc, ios, const, dram, b1_sb, b2_sb, rep):
    thr_pos = const.tile([P, 2], F32, tag=f"thrp{rep}")
    thr_neg = const.tile([P, 2], F32, tag=f"thrn{rep}")
    _emit_scale_phase(nc, tc, ios, const, dram, thr_pos, thr_neg, rep)
    t1p, t1n = thr_pos[:, 0:1], thr_neg[:, 0:1]
    t2p, t2n = thr_pos[:, 1:2], thr_neg[:, 1:2]

    hT = dram.tile([D_FF, M_CORE], HALF, tag=f"hT{rep}")
    w2qT = dram.tile([D_FF, D_MODEL], HALF, tag=f"w2qT{rep}")

    # ---------------- fc1 (+ interleaved w2 quant) ----------------
    with ExitStack() as fc1_scope:
        stage = fc1_scope.enter_context(tc.tile_pool(name=f"kxm_stage{rep}", bufs=3))
        kxmq = fc1_scope.enter_context(tc.tile_pool(name=f"kxmq{rep}", bufs=8))
        qtmp = fc1_scope.enter_context(tc.tile_pool(name=f"qtmp{rep}", bufs=3))
        # holds ALL of xT (4 k-tiles x 4 n-tiles of [128,4,512] fp16 = 64KB/par)
        kxn1 = fc1_scope.enter_context(tc.tile_pool(name=f"kxn1{rep}", bufs=17))
        w2stage = fc1_scope.enter_context(tc.tile_pool(name=f"w2stage{rep}", bufs=2))
        w2tmp = fc1_scope.enter_context(tc.tile_pool(name=f"w2tmp{rep}", bufs=2))
        w2qsb = fc1_scope.enter_context(tc.tile_pool(name=f"w2qsb{rep}", bufs=2))

        # w2 ternarization -> DRAM. One block per fc1 kxm-producer call so its
        # DVE work interleaves 1:1 with fc1's w1 quant instead of hogging the
        # engine up front or trailing behind fc1.
        w2_state = {"blk": 0}

        def emit_w2_block():
            blk = w2_state["blk"]
            if blk >= D_FF // P:
                return
            w2_state["blk"] = blk + 1
            s = w2stage.tile([P, D_MODEL], F32)
            nc.sync.dma_start(out=s[:], in_=ios["w2T"].ap()[blk * P : (blk + 1) * P, :])
            q = w2qsb.tile([P, D_MODEL], HALF)
            a = w2tmp.tile([P, D_MODEL], HALF)
            nc.vector.tensor_scalar(q[:], s[:], t2p, None, IS_GE)
            nc.vector.tensor_scalar(a[:], s[:], t2n, None, IS_LE)
            nc.vector.tensor_sub(q[:], q[:], a[:])
            nc.sync.dma_start(out=w2qT[blk * P : (blk + 1) * P, :], in_=q[:])

        base_producer, kxm_shape = dma_from_dram_kxm(stage, ios["w1T"].ap())

        def kxm_q_producer(nc_, md):
            t32 = base_producer(nc_, md)
            q = kxmq.tile([P, md.k_subtiles, md.m_tile], HALF, tag="kxmq")
            a = qtmp.tile([P, md.k_subtiles, md.m_tile], HALF, tag="qtmp")
            nc_.vector.tensor_scalar(q[:], t32[:], t1p, None, IS_GE)
            nc_.vector.tensor_scalar(a[:], t32[:], t1n, None, IS_LE)
            nc_.vector.tensor_sub(q[:], q[:], a[:])
            emit_w2_block()
            return q

        base_kxn_producer, kxn_shape = dma_from_dram_kxn(kxn1, ios["xT"].ap())

        # xT is only 8MB/core in fp16 — memoize tiles so each (k, n) block is
        # DMA'd exactly once and lives in SBUF for all 16 m-stripes (the
        # composable would otherwise re-load each n-stripe per m-stripe).
        xt_memo = {}

        def kxn_producer(nc_, md):
            key = (md.k_tile_idx, md.n_tile_idx)
            if key not in xt_memo:
                xt_memo[key] = base_kxn_producer(nc_, md)
            return xt_memo[key]

        def fc1_reducer(nc_, psum, sbuf, md):
            j = md.m_tile_idx * md.m_subtiles + md.m_subtile_idx
            nc_.scalar.activation(sbuf, psum, GELU, bias=b1_sb[:, j : j + 1])

        composable_matmul_tile_kernel(
            tc=tc,
            kxm_shape=kxm_shape,
            kxn_shape=kxn_shape,
            output_type=HALF,
            kxm_producer=kxm_q_producer,
            kxn_producer=kxn_producer,
            mxn_consumer=dma_to_dram_mxn(hT[:]),
            mxn_subtile_reducer=fc1_reducer,
            psum_n_bufs=2,
        )

        # drain any w2 blocks not covered by producer calls
        while w2_state["blk"] < D_FF // P:
            emit_w2_block()

    # ---------------- fc2 ----------------
    with ExitStack() as fc2_scope:
        kxm2 = fc2_scope.enter_context(tc.tile_pool(name=f"kxm2{rep}", bufs=18))
        kxn2 = fc2_scope.enter_context(tc.tile_pool(name=f"kxn2{rep}", bufs=18))

        kxm2_producer, kxm2_shape = dma_from_dram_kxm(kxm2, w2qT[:])
        kxn2_producer, kxn2_shape = dma_from_dram_kxn(kxn2, hT[:])

        def fc2_reducer(nc_, psum, sbuf, md):
            j = md.m_tile_idx * md.m_subtiles + md.m_subtile_idx
            nc_.any.tensor_scalar_add(sbuf, psum, b2_sb[:, j : j + 1])

        composable_matmul_tile_kernel(
            tc=tc,
            kxm_shape=kxm2_shape,
            kxn_shape=kxn2_shape,
            output_type=F32,
            kxm_producer=kxm2_producer,
            kxn_producer=kxn2_producer,
            mxn_consumer=dma_to_dram_mxn(ios["outT"].ap()),
            mxn_subtile_reducer=fc2_reducer,
            psum_n_bufs=2,
        )


def _build_nc(repeats=1):
    nc = bacc.Bacc("TRN2", target_bir_lowering=False, debug=False, num_devices=N_CORES)

    ios = {
        "xT": nc.declare_dram_parameter("xT", [D_MODEL, M_CORE], HALF, isOutput=False),
        "w1T": nc.declare_dram_parameter("w1T", [D_MODEL, D_FF], F32, isOutput=False),
        "w2T": nc.declare_dram_parameter("w2T", [D_FF, D_MODEL], F32, isOutput=False),
        "w1s": nc.declare_dram_parameter("w1s", [P, SLICE_F], F32, isOutput=False),
        "w2s": nc.declare_dram_parameter("w2s", [P, SLICE_F], F32, isOutput=False),
        "b1": nc.declare_dram_parameter("b1", [D_FF], F32, isOutput=False),
        "b2": nc.declare_dram_parameter("b2", [D_MODEL], F32, isOutput=False),
        "outT": nc.declare_dram_parameter(
            "outT", [D_MODEL, M_CORE], F32, isOutput=True
        ),
    }

    with tile.TileContext(nc) as tc, ExitStack() as top:
        const = top.enter_context(tc.tile_pool(name="const", bufs=1))
        dram = top.enter_context(tc.tile_pool(name="dram", bufs=1, space="DRAM"))

        b1_sb = const.tile([P, D_FF // P], F32)
        nc.sync.dma_start(
            out=b1_sb[:], in_=ios["b1"].ap().rearrange("(a p) -> p a", p=P)
        )
        b2_sb = const.tile([P, D_MODEL // P], F32)
        nc.sync.dma_start(
            out=b2_sb[:], in_=ios["b2"].ap().rearrange("(a p) -> p a", p=P)
        )

        for rep in range(repeats):
            _emit_pipeline(nc, tc, ios, const, dram, b1_sb, b2_sb, rep)

    nc.compile()
    return nc


def _get_nc(repeats=1):
    if repeats not in _BUILD_CACHE:
        _BUILD_CACHE[repeats] = _build_nc(repeats)
    return _BUILD_CACHE[repeats]


def _prepare_in_maps(x, w1, b1, w2, b2):
    x = np.asarray(x, dtype=np.float32)
    w1 = np.asarray(w1, dtype=np.float32)
    w2 = np.asarray(w2, dtype=np.float32)
    b1 = np.asarray(b1, dtype=np.float32)
    b2 = np.asarray(b2, dtype=np.float32)

    x2 = x.reshape(M_TOTAL, D_MODEL)
    w1T = np.ascontiguousarray(w1.T)  # [D_MODEL, D_FF] f32
    w2T = np.ascontiguousarray(w2.T)  # [D_FF, D_MODEL] f32
    w1sl = w1.reshape(N_CORES, P, SLICE_F)
    w2sl = w2.reshape(N_CORES, P, SLICE_F)

    in_maps = []
    for c in range(N_CORES):
        shard = x2[c * M_CORE : (c + 1) * M_CORE]
        xT_c = np.ascontiguousarray(shard.T).astype(np.float16)
        in_maps.append(
            {
                "xT": xT_c,
                "w1T": w1T,
                "w2T": w2T,
                "w1s": w1sl[c],
                "w2s": w2sl[c],
                "b1": b1,
                "b2": b2,
            }
        )
    return in_maps


def _assemble(res):
    outT_full = np.concatenate(
        [res.results[c]["outT"] for c in range(N_CORES)], axis=1
    )  # [D_MODEL, M_TOTAL]
    out = np.ascontiguousarray(outT_full.T).reshape(4, 4096, D_MODEL)
    return out.astype(np.float32, copy=False)


def kernel(x, w1, b1, w2, b2):
    nc = _get_nc()
    in_maps = _prepare_in_maps(x, w1, b1, w2, b2)
    res = run_bass_kernel_spmd(nc, in_maps, list(range(N_CORES)))
    return _assemble(res)


if __name__ == "__main__":
    rng = np.random.default_rng(0)
    x = rng.standard_normal((4, 4096, D_MODEL), dtype=np.float32)
    w1 = rng.standard_normal((D_FF, D_MODEL), dtype=np.float32)
    w2 = rng.standard_normal((D_MODEL, D_FF), dtype=np.float32)
    out = kernel(
        x=x,
        w1=w1,
        b1=np.zeros(D_FF, np.float32),
        w2=w2,
        b2=np.zeros(D_MODEL, np.float32),
    )
    print(out.shape, out.dtype)
